# revision 22
# baseline (speedup 1.0000x reference)
"""Trainium2 Bass kernel for nn_Attention_81750407512209.

Full attention: out = softmax((x Wq)(x Wk)^T / sqrt(128)) @ (x Wv)
B=8 batches sharded 1:1 onto 8 NeuronCores (data parallel, weights replicated).

Two-pass design (N=4096 ctx, D=128), per core:
  - Prologue: x^T via PE transpose; Q^T/K^T projections in float32r then
    stored bf16 (1/sqrt(128) folded into Wq); V in [kv,d] layout bf16 with
    a ones column appended (row sums accumulate in PSUM col 128 during AV).
  - Main loop, software-pipelined over q-tiles (tile j):
      iter j:   pass-1 scores (4x1024 PSUM chunks) -> DVE row-max per chunk,
                slot freed immediately (maxes are off the recycle path).
      iter j+1: pass-2 recomputes the identical score chunks -> ScalarE exp
                with the full-row bias (min of the 4 negated chunk maxes,
                ready since iter j) -> P bf16 in SBUF.  No flash rescales:
                single exact bias per row.
      iter j+2: AV: 32 back-to-back bf16 matmuls lhsT=P^T tile, rhs=V||1;
                DVE reciprocal of col 128, ScalarE normalize, store.
  - P^T via xbar DMA transposes on the sync HWDGE engine only (serialized
    resource; dual-engine issue corrupts data - measured previously).
  - PE streams pass-2 chunk, pass-1 chunk, 8 AV matmuls per sub-step: ~93%
    duty keeps the PE at full p-state; scores are recomputed (PE is cheap)
    so no PSUM chunk ever waits on its own max before exp.
"""

import numpy as np
from contextlib import ExitStack

import concourse.bass as bass
import concourse.tile as tile
from concourse import bacc, mybir
from concourse.bass_utils import run_bass_kernel_spmd
from concourse.masks import make_identity

F32 = mybir.dt.float32
F32R = mybir.dt.float32r
BF16 = mybir.dt.bfloat16
AX = mybir.AxisListType.X
OP = mybir.AluOpType
AF = mybir.ActivationFunctionType

B, N, D = 8, 4096, 128
NT = N // 128                    # 32 kv/q tiles
W = 1024                         # chunk width
NCH = N // W                     # 4 chunks per tile
SCALE = 1.0 / np.sqrt(np.float32(D))


def build_attention(nc: bacc.Bacc):
    x = nc.dram_tensor("x", [N, D], F32, kind="ExternalInput").ap()
    wq = nc.dram_tensor("w_query", [D, D], F32, kind="ExternalInput").ap()
    wk = nc.dram_tensor("w_key", [D, D], F32, kind="ExternalInput").ap()
    wv = nc.dram_tensor("w_value", [D, D], F32, kind="ExternalInput").ap()
    out = nc.dram_tensor("out", [N, D], F32, kind="ExternalOutput").ap()

    with tile.TileContext(nc) as tc, ExitStack() as ctx:
        consts = ctx.enter_context(tc.tile_pool(name="consts", bufs=1))
        big = ctx.enter_context(tc.tile_pool(name="big", bufs=1))
        xin = ctx.enter_context(tc.tile_pool(name="xin", bufs=8))
        pbuf = ctx.enter_context(tc.tile_pool(name="pbuf", bufs=4))
        stats = ctx.enter_context(tc.tile_pool(name="stats", bufs=4))
        ostage = ctx.enter_context(tc.tile_pool(name="ostage", bufs=4))

        ident = consts.tile([128, 128], F32, name="ident")
        make_identity(nc, ident[:])

        wq_st = consts.tile([128, 128], F32, name="wq_st")
        wk_st = consts.tile([128, 128], F32, name="wk_st")
        wv_st = consts.tile([128, 128], F32, name="wv_st")
        nc.sync.dma_start(wq_st[:], wq[:])
        nc.sync.dma_start(wk_st[:], wk[:])
        nc.sync.dma_start(wv_st[:], wv[:])
        wq_r = consts.tile([128, 128], F32R, name="wq_r")
        wk_r = consts.tile([128, 128], F32R, name="wk_r")
        wv_r = consts.tile([128, 128], F32R, name="wv_r")
        nc.vector.tensor_scalar_mul(wq_r[:], wq_st[:], float(SCALE))
        nc.vector.tensor_copy(wk_r[:], wk_st[:])
        nc.vector.tensor_copy(wv_r[:], wv_st[:])

        xT = big.tile([128, N], F32R, name="xT")
        kT = big.tile([128, N], BF16, name="kT")
        qT = big.tile([128, N], BF16, name="qT")
        vaug = big.tile([128, NT, 129], BF16, name="vaug")
        nc.gpsimd.memset(vaug[:, :, 128:129], 1.0)

        # ---- prologue: x^T, projections (scoped PSUM pool) ----
        with tc.tile_pool(name="ps_pro", bufs=2, space="PSUM") as ps_pro:
            for c in range(N // 512):
                sl = slice(c * 512, (c + 1) * 512)
                for u in range(4):
                    i = c * 4 + u
                    xt = xin.tile([128, 128], F32, tag="xt", name="xt")
                    nc.gpsimd.dma_start(xt[:], x[i * 128:(i + 1) * 128, :])
                    ps = ps_pro.tile([128, 128], F32, tag="xtp", name="xtp")
                    nc.tensor.transpose(ps[:], xt[:], ident[:])
                    if i % 2 == 0:
                        nc.vector.tensor_copy(xT[:, i * 128:(i + 1) * 128], ps[:])
                    else:
                        nc.scalar.copy(xT[:, i * 128:(i + 1) * 128], ps[:])
                pk = ps_pro.tile([128, 512], F32, tag="proj", name="pk")
                nc.tensor.matmul(pk[:], wk_r[:], xT[:, sl], start=True, stop=True)
                nc.vector.tensor_copy(kT[:, sl], pk[:])
                pq = ps_pro.tile([128, 512], F32, tag="proj", name="pq")
                nc.tensor.matmul(pq[:], wq_r[:], xT[:, sl], start=True, stop=True)
                nc.scalar.copy(qT[:, sl], pq[:])
                for u in range(4):
                    i = c * 4 + u
                    pv = ps_pro.tile([128, 128], F32, tag="vproj", name="pv")
                    nc.tensor.matmul(
                        pv[:], xT[:, i * 128:(i + 1) * 128], wv_r[:],
                        start=True, stop=True,
                    )
                    nc.scalar.copy(vaug[:, i, 0:128], pv[:])

        # ---- main loop PSUM: 3x1024-col score slots + 2 AV accumulators ----
        ps_s = ctx.enter_context(tc.tile_pool(name="ps_s", bufs=3, space="PSUM"))
        ps_av = ctx.enter_context(tc.tile_pool(name="ps_av", bufs=2, space="PSUM"))

        def score_chunk(j, c):
            # chunk c of tile j: S[q-tile j, kv W*c : W*(c+1)] via 2 matmuls
            s = ps_s.tile([128, W], F32, tag="sh", name="sh")
            qsl = qT[:, j * 128:(j + 1) * 128]
            for k in range(W // 512):
                off = c * W + k * 512
                nc.tensor.matmul(
                    s[:, k * 512:(k + 1) * 512],
                    qsl,
                    kT[:, off:off + 512],
                    start=True,
                    stop=True,
                )
            return s

        # pipeline state per tile: nmx [128, NCH] negated chunk maxes,
        # m [128,1] full-row negated max, P/PT, av
        st1 = None   # tile j-1: (nmx,) -> will exp this iter
        st2 = None   # tile j-2: (P, PT, av?) -> AV + finish this iter
        e1 = None    # tile j-1 exp products (P, PT, m)
        e2 = None
        for it in range(NT + 2):
            j2 = it - 2   # AV/finish tile
            j1 = it - 1   # exp tile
            j0 = it       # pass-1 tile

            if 0 <= j1 < NT:
                P = pbuf.tile([128, N], BF16, tag="P", name="P")
                PT = pbuf.tile([128, NT, 128], BF16, tag="PT", name="PT")
                m1 = st1  # [128,1] bias for tile j1
            if j0 < NT:
                nmx = stats.tile([128, NCH], F32, tag="nmx", name="nmx")
            if j2 >= 0:
                P2, PT2 = e1
                av = ps_av.tile([128, 129], F32, tag="av", name="av")

            for c in range(NCH):
                # pass-2: recompute chunk of tile j1, exp it promptly
                if 0 <= j1 < NT:
                    s2c = score_chunk(j1, c)
                    nc.scalar.activation(
                        P[:, c * W:(c + 1) * W], s2c[:], AF.Exp, bias=m1[:]
                    )
                # pass-1: chunk of tile j0, row-max only
                if j0 < NT:
                    s1c = score_chunk(j0, c)
                    nc.vector.reduce_max(
                        nmx[:, c:c + 1], s1c[:], axis=AX, negate=True
                    )
                # AV for tile j2: 8 kv-tiles per sub-step, single accumulation
                if j2 >= 0:
                    for t in range(8 * c, 8 * (c + 1)):
                        nc.tensor.matmul(
                            av[:], PT2[:, t, :], vaug[:, t, :],
                            start=(t == 0), stop=(t == NT - 1),
                        )
                # transposes: after exp chunks 1 and 3 land
                if 0 <= j1 < NT and c == 1:
                    nc.sync.dma_start_transpose(
                        PT[:, 0:16, :], P[:, 0:2048]
                    )
                if 0 <= j1 < NT and c == 3:
                    nc.sync.dma_start_transpose(
                        PT[:, 16:NT, :], P[:, 2048:N]
                    )

            # full-row bias for tile j0 (consumed next iteration)
            if j0 < NT:
                m = stats.tile([128, 1], F32, tag="m", name="m")
                nc.vector.tensor_reduce(m[:], nmx[:], axis=AX, op=OP.min)

            # finish tile j2: reciprocal of row sums, normalize, store
            if j2 >= 0:
                linv = stats.tile([128, 1], F32, tag="linv", name="linv")
                nc.vector.reciprocal(linv[:], av[:, 128:129])
                ost = ostage.tile([128, 128], F32, tag="ost", name="ost")
                nc.vector.tensor_scalar_mul(ost[:], av[:, 0:128], linv[:])
                nc.gpsimd.dma_start(out[j2 * 128:(j2 + 1) * 128, :], ost[:])

            st1 = m if j0 < NT else None
            e1 = (P, PT) if 0 <= j1 < NT else None

    nc.compile()
    return nc


_NC_CACHE = {}


def _get_nc():
    if "nc" not in _NC_CACHE:
        nc = bacc.Bacc("TRN2", target_bir_lowering=False, debug=False, num_devices=B)
        _NC_CACHE["nc"] = build_attention(nc)
    return _NC_CACHE["nc"]


def kernel(x, w_query, w_key, w_value, _trace=False):
    x = np.ascontiguousarray(np.asarray(x, dtype=np.float32))
    w_query = np.ascontiguousarray(np.asarray(w_query, dtype=np.float32))
    w_key = np.ascontiguousarray(np.asarray(w_key, dtype=np.float32))
    w_value = np.ascontiguousarray(np.asarray(w_value, dtype=np.float32))
    nc = _get_nc()
    in_maps = [
        {"x": x[b], "w_query": w_query, "w_key": w_key, "w_value": w_value}
        for b in range(B)
    ]
    res = run_bass_kernel_spmd(nc, in_maps, core_ids=list(range(B)), trace=_trace)
    out_full = np.stack([res.results[b]["out"] for b in range(B)])
    if _trace:
        kernel.last_exec_time_ns = res.exec_time_ns
        if res.instructions_and_trace is not None:
            kernel.last_trace_path = res.instructions_and_trace[1]
    return out_full
